# revision 21
# baseline (speedup 1.0000x reference)
"""Trainium2 Bass kernel for nn_Loss_20933670601009 (gathered-prob NLL loss).

Strategy: the loss only touches 3 elements per (l, b) position (one gathered
prob from each of rule/token/reference tables). rule/token probs are element-
gathered straight from HBM with 8 indirect DMAs (the SWDGE primitive consumes
one offset per partition row, 128 per instruction). The small reference table
(512 wide, 1MB per core) is instead streamed whole via HWDGE — overlapping
the gathers — and its elements extracted on the DVE with a host-prebuilt
one-hot mask (multiply + segmented reduce).

All index arithmetic happens on the host: flat offsets are precomputed from
gt; invalid components (gt=-1) redirect to an appended 0.0 element; masked
positions redirect rule->1.0 / others->0.0 (ref mask block all-zero) so
log(1.0)=0 contributes nothing — no validity or mask ops on device.

Hand-scheduled raw Block (no Tile framework). Same-engine dependent ops are
sem-interlocked (the engine pipelines reads of the next op under writes of
the previous — hazard confirmed on HW). Indirect-DMA completion sems fire
before SBUF writes are visible, so a gpsimd drain gates the consumers.

Sharding: data-parallel over L_a (128 rows -> 16 x 8 cores, 512 positions
per core). Host sums the 8 per-core scalars.
"""

import os
import sys

import ml_dtypes
import numpy as np

for _p in ("/opt/trn_rl_repo", "/root/.axon_site/_ro/trn_rl_repo"):
    if os.path.isdir(_p) and _p not in sys.path:
        sys.path.insert(0, _p)

L_A, B = 128, 32
V_RULE, V_TOK, V_REF = 2048, 32000, 512
EPS = 1e-07
N_CORES = 8
L_SH = L_A // N_CORES            # 16 sequence rows per core
NPOS = L_SH * B                  # 512 positions per core
P = 128
J = NPOS // P                    # 4 positions per partition
SEG = (0, NPOS * V_RULE)         # rule, token segments in flat
N_FLAT = NPOS * (V_RULE + V_TOK)
ONE_IDX = N_FLAT                 # flat[ONE_IDX] = 1.0 (masked positions, rule slot)
ZERO_IDX = N_FLAT + 1            # flat[ZERO_IDX] = 0.0 (invalid / masked others)

_CACHE = {}


def _build():
    """Build + compile the per-core Bass module (same NEFF on all 8 cores)."""
    import concourse.bacc as bacc
    import concourse.bass as bass
    import concourse.mybir as mybir

    f32 = mybir.dt.float32
    bf16 = mybir.dt.bfloat16
    i32 = mybir.dt.int32

    nc = bacc.Bacc(
        "TRN2",
        target_bir_lowering=False,
        debug=False,
        enable_asserts=False,
        num_devices=N_CORES,
        detect_race_conditions=False,
    )

    # meta (int32 [128, 16]):
    #   cols 0:8   precomputed flat gather offsets, component-major blocks
    #              of 4 (rule|token): col = 4*c + j addresses position q = p*4+j
    #   col  12    -1/B as f32 bit pattern (PE reduce weights)
    #   col  13    EPS as f32 bit pattern (activation bias)
    meta_d = nc.dram_tensor("meta", [P, 16], i32, kind="ExternalInput")
    flat_d = nc.dram_tensor("probs_flat", [N_FLAT + 2, 1], f32, kind="ExternalInput")
    # ref rows + one-hot mask, concatenated: cols 0:2048 data, 2048:4096 mask
    # (refcat[p, 512*j + v] = ref[q = p*4 + j, v])
    refc_d = nc.dram_tensor("refcat", [P, 2 * J * V_REF], bf16, kind="ExternalInput")
    out_d = nc.dram_tensor("out", [1, 1], f32, kind="ExternalOutput")

    with (
        nc.semaphore("io") as io,
        nc.semaphore("g") as g,
        nc.semaphore("cv") as cv,
        nc.semaphore("v1") as v1,
        nc.semaphore("oo") as oo,
        nc.sbuf_tensor("meta_sb", [P, 16], i32) as meta,
        nc.sbuf_tensor("refc_sb", [P, 2 * J * V_REF], bf16) as refc,
        nc.sbuf_tensor("gv", [P, 12], f32) as gv,
        nc.sbuf_tensor("warm", [P, 1], i32) as warm,
        nc.sbuf_tensor("wout", [P, 1], f32) as wout,
        nc.sbuf_tensor("s", [P, J], f32) as s,
        nc.sbuf_tensor("ln", [P, J], f32) as ln,
        nc.sbuf_tensor("rs", [P, 1], f32) as rs,
        nc.sbuf_tensor("res", [1, 1], f32) as res,
        nc.psum_tensor("acc", [1, 1], f32) as acc,
    ):
        with nc.Block() as block:

            @block.sync
            def _(sync):
                sync.dma_start(meta[:], meta_d[:]).then_inc(io, 16)
                sync.dma_start(refc[:], refc_d[:]).then_inc(io, 16)

            @block.gpsimd
            def _(gp):
                # warm the SWDGE path while the meta DMA is in flight: the
                # first Pool DMA instruction pays ~0.3us extra; absorb it on
                # a dummy gather from flat[0] (offsets memset to zero)
                gp.memset(warm[:], 0)
                gp.indirect_dma_start(
                    out=wout[:],
                    out_offset=None,
                    in_=flat_d[:],
                    in_offset=bass.IndirectOffsetOnAxis(ap=warm[:], axis=0),
                ).then_inc(g, 16)
                gp.wait_ge(io, 16)
                for col in range(8):
                    gp.indirect_dma_start(
                        out=gv[:, col:col + 1],
                        out_offset=None,
                        in_=flat_d[:],
                        in_offset=bass.IndirectOffsetOnAxis(
                            ap=meta[:, col:col + 1], axis=0
                        ),
                    ).then_inc(g, 16)
                # indirect-DMA completion sems fire before the SBUF writes
                # are visible; drain the queues, then release the consumers
                gp.drain()
                gp.sem_inc(cv, 1)

            @block.vector
            def _(v):
                X = mybir.AxisListType.X
                v.wait_ge(io, 32)
                v.tensor_mul(
                    out=refc[:, 0:J * V_REF], in0=refc[:, 0:J * V_REF],
                    in1=refc[:, J * V_REF:],
                ).then_inc(v1, 1)
                v.wait_ge(v1, 1)
                # ref per-position sums land in gv cols 8:12 (gathers fill 0:8)
                v.reduce_sum(
                    out=gv[:, 8:12],
                    in_=refc[:, 0:J * V_REF].rearrange("p (j e) -> p j e", e=V_REF),
                    axis=X,
                ).then_inc(v1, 1)
                v.wait_ge(cv, 1)
                v.wait_ge(v1, 2)
                # s[p, j] = sum_c gv[p, 4c+j] in ONE op (strided innermost)
                v.reduce_sum(
                    out=s[:],
                    in_=gv[:].rearrange("p (c j) -> p j c", j=J),
                    axis=X,
                ).then_inc(cv, 1)

            @block.scalar
            def _(sc):
                sc.wait_ge(cv, 2)
                # ln = Ln(s + eps); rs[p] = sum_j ln[p, j]
                # (log(x+eps) ~ torch's x + (x<eps)*eps to ~1e-7 absolute)
                sc.activation(
                    out=ln[:], in_=s[:], func=mybir.ActivationFunctionType.Ln,
                    bias=meta[:, 13:14].bitcast(f32), accum_out=rs[:],
                ).then_inc(cv, 1)

            @block.tensor
            def _(t):
                t.wait_ge(cv, 3)
                # partition reduction via PE; weight -1/B folds negation + mean
                t.matmul(
                    out=acc[:], lhsT=rs[:], rhs=meta[:, 12:13].bitcast(f32),
                    start=True, stop=True,
                ).then_inc(cv, 1)

            @block.scalar
            def _(sc):
                sc.wait_ge(cv, 4)
                sc.copy(out=res[:], in_=acc[:]).then_inc(cv, 1)

            @block.sync
            def _(sync):
                sync.wait_ge(cv, 5)
                sync.dma_start(out_d[:], res[:]).then_inc(oo, 16)

    nc.compile()
    return nc


def get_nc():
    if "nc" not in _CACHE:
        _CACHE["nc"] = _build()
    return _CACHE["nc"]


def make_in_maps(rule_probs, token_probs, reference_probs, ground_truth_actions, mask):
    """Shard the full inputs into 8 per-core input maps."""
    rule_probs = np.ascontiguousarray(np.asarray(rule_probs, dtype=np.float32))
    token_probs = np.ascontiguousarray(np.asarray(token_probs, dtype=np.float32))
    reference_probs = np.ascontiguousarray(np.asarray(reference_probs, dtype=np.float32))
    gt = np.asarray(ground_truth_actions, dtype=np.int32)
    mask_in = np.asarray(mask, dtype=np.int32)

    q = np.arange(NPOS, dtype=np.int64)
    tail = np.array([1.0, 0.0], dtype=np.float32)
    scal = np.empty(2, np.float32)
    scal[0] = -1.0 / B
    scal[1] = EPS
    scal_i = scal.view(np.int32)
    VS = (V_RULE, V_TOK)

    in_maps = []
    for i in range(N_CORES):
        lo, hi = i * L_SH, (i + 1) * L_SH
        gt_sh = gt[lo:hi].reshape(NPOS, 3)
        m = mask_in[lo:hi].reshape(NPOS) != 0

        meta = np.zeros((P, 16), np.int32)
        for c in range(2):
            idx = gt_sh[:, c].astype(np.int64)
            off = SEG[c] + q * VS[c] + idx
            # invalid component -> 0.0 element
            off = np.where(idx >= 0, off, ZERO_IDX)
            # masked-out position -> rule reads 1.0, token reads 0.0
            off = np.where(m, off, ONE_IDX if c == 0 else ZERO_IDX)
            meta[:, c * 4:(c + 1) * 4] = off.reshape(P, J).astype(np.int32)
        meta[:, 12] = scal_i[0]
        meta[:, 13] = scal_i[1]

        # ref path: streamed rows + one-hot mask (validity & batch mask folded)
        refc = np.zeros((P, 2 * J * V_REF), ml_dtypes.bfloat16)
        refc[:, 0:J * V_REF] = (
            reference_probs[lo:hi].reshape(P, J * V_REF).astype(ml_dtypes.bfloat16))
        gf = gt_sh[:, 2].astype(np.int64)
        sel = (gf >= 0) & m
        pos = np.where(sel, np.maximum(gf, 0), 0)
        refk = np.zeros((NPOS, V_REF), ml_dtypes.bfloat16)
        refk[np.arange(NPOS)[sel], pos[sel]] = 1.0
        refc[:, J * V_REF:] = refk.reshape(P, J * V_REF)

        probs_flat = np.concatenate(
            [
                rule_probs[lo:hi].reshape(-1),
                token_probs[lo:hi].reshape(-1),
                tail,
            ]
        )
        in_maps.append({
            "meta": meta, "probs_flat": probs_flat.reshape(-1, 1),
            "refcat": refc,
        })
    return in_maps


def run(inputs, trace=False, trace_cores=None):
    """Run on the 8 NeuronCores; returns (scalar ndarray, BassKernelResults)."""
    from concourse.bass_utils import run_bass_kernel_spmd

    nc = get_nc()
    in_maps = make_in_maps(**inputs)
    res = run_bass_kernel_spmd(
        nc,
        in_maps,
        core_ids=list(range(N_CORES)),
        trace=trace,
        trace_cores=trace_cores,
    )
    total = np.float64(0.0)
    for r in res.results:
        total += np.float64(r["out"].reshape(())[()])
    return np.asarray(total, dtype=np.float32), res


def kernel(**inputs) -> np.ndarray:
    out, _ = run(inputs)
    return out
